# revision 29
# baseline (speedup 1.0000x reference)
"""Trainium2 Bass kernel for nn_EpisodicMemory (modularity + conductance).

Per batch element (N=2048 rows, D=512 dims):
    S = rep @ rep.T            (never materialized!)
    S' = S / max(||S_row||, 1e-12)
    communities = contiguous runs given by cumsum(boundaries)
    mod  = (sum_same S' - sum_c D_c^2/total) / total
    cond = mean_c (D_c - W_c)/(W_c + D_c + 1e-10)

S-free formulation (everything is exact-fp32-grade):
    G = rep^T rep (512x512 Gram);  H = rep @ G;  ssq_i = <rep_i, H_i>
    rowsum_i = <rep_i, u>,  u = sum_j rep_j
    q_i = sum_{j in comm(i)} S_ij = <rep_i, R(c_i)> via forward+reverse
          segmented scans over rep^T plus a ones-matvec partition reduce
          (minus the double-counted self term ||rep_i||^2).
    W_c, D_c from segmented scans of rnorm*q and rnorm*rowsum in a
    (16,128) layout with cross-partition carry fix-up.

Sharding: data-parallel over the batch axis, one batch element per core,
8 NeuronCores. Full inputs in, full (2, 8) output out.
"""
import sys
if '/opt/trn_rl_repo' not in sys.path:
    sys.path.insert(0, '/opt/trn_rl_repo')

import numpy as np

N = 2048
D = 512
NT = N // 128          # 16 row tiles of rep
ND = D // 128          # 4 partition chunks of repT
NJ = N // 512          # 4 free chunks of 512
EPS_NORM = 1e-12
EPS_COND = 1e-10

_COMPILED = None


def _build():
    import concourse.bacc as bacc
    import concourse.tile as tile
    from concourse import mybir
    from concourse.masks import make_identity

    f32 = mybir.dt.float32
    f32r = mybir.dt.float32r
    i32 = mybir.dt.int32
    Alu = mybir.AluOpType
    Act = mybir.ActivationFunctionType

    nc = bacc.Bacc("TRN2", target_bir_lowering=False, debug=False)
    rep_d = nc.dram_tensor("rep", [N, D], f32, kind="ExternalInput")
    bnd_d = nc.dram_tensor("bnd", [N], i32, kind="ExternalInput")
    out_d = nc.dram_tensor("out", [1, 2], f32, kind="ExternalOutput")

    rep_tiles_d = rep_d.rearrange("(t p) d -> t p d", p=128)
    b_row_d = bnd_d.rearrange("(a f) -> a f", a=1)

    with tile.TileContext(nc) as tc:
        with (
            tc.tile_pool(name="big", bufs=1) as big,
            tc.tile_pool(name="small", bufs=1) as small,
            tc.tile_pool(name="scr", bufs=4) as scrp,
            tc.tile_pool(name="rows", bufs=3) as rows,
            tc.tile_pool(name="revp", bufs=2) as revp,
            tc.tile_pool(name="pmm", bufs=5, space="PSUM") as pmm,
            tc.tile_pool(name="psm", bufs=3, space="PSUM") as psm,
        ):
            # ---------- constants ----------
            ident = small.tile([128, 128], f32)
            make_identity(nc, ident[:])
            ones_col = small.tile([128, 1], f32)
            nc.vector.memset(ones_col[:], 1.0)

            # ---------- load inputs (issue split over 2 HWDGE queues) ----
            b_row = small.tile([1, N], i32)
            nc.sync.dma_start(b_row[:], b_row_d[:])
            rep = []
            for t in range(NT):
                rt = big.tile([128, D], f32, tag=f"rep{t}")
                eng = nc.sync if t % 2 == 0 else nc.scalar
                eng.dma_start(rt[:], rep_tiles_d[t])
                rep.append(rt)

            # ---------- masks ----------
            bf_row = rows.tile([1, N], f32, tag="rowbuf")
            nc.scalar.activation(bf_row[:], b_row[:], Act.Copy)
            m_row = rows.tile([1, N], f32, tag="rowbuf")   # 0 at starts
            nc.scalar.activation(m_row[:], bf_row[:], Act.Copy,
                                 bias=1.0, scale=-1.0)
            l_row = rows.tile([1, N], f32, tag="rowbuf")   # 1 at ends
            nc.vector.memset(l_row[:, N-1:N], 1.0)
            nc.scalar.activation(l_row[:, 0:N-1], bf_row[:, 1:N], Act.Copy)
            mp_row = rows.tile([1, N], f32, tag="rowbuf")  # 0 at ends
            nc.scalar.activation(mp_row[:], l_row[:], Act.Copy,
                                 bias=1.0, scale=-1.0)

            m_td = small.tile([16, 128], f32)
            nc.sync.dma_start(m_td[:], m_row.rearrange("a (p f) -> a p f", p=16))
            l_td = small.tile([16, 128], f32)
            nc.sync.dma_start(l_td[:], l_row.rearrange("a (p f) -> a p f", p=16))

            Pm = small.tile([16, 128], f32)
            nc.vector.tensor_tensor_scan(out=Pm[:], data0=m_td[:],
                                         data1=m_td[:], initial=1.0,
                                         op0=Alu.mult, op1=Alu.bypass)
            bch_ps = psm.tile([1, 16], f32, tag="sm")
            nc.tensor.transpose(bch_ps[:], Pm[:, 127:128], ident[:16, :16])
            bch_row = small.tile([1, 16], f32)
            nc.vector.tensor_copy(bch_row[:], bch_ps[:])

            # ---------- transpose rep -> repT (PE; PSUM->SBUF on ACT) ----
            repT = []
            for dc in range(ND):
                rT = big.tile([128, N], f32, tag=f"repT{dc}")
                repT.append(rT)
            for dc in range(ND):
                for tg in range(4):
                    tp_ps = pmm.tile([128, 512], f32, tag="mm")
                    for tt in range(4):
                        t = tg * 4 + tt
                        nc.tensor.transpose(
                            tp_ps[:, tt*128:(tt+1)*128],
                            rep[t][:, dc*128:(dc+1)*128], ident[:])
                    nc.scalar.copy(repT[dc][:, tg*512:(tg+1)*512], tp_ps[:])

            # ---------- segmented scans over repT (DVE) + P (gpsimd) ----
            fwd = []
            for dc in range(ND):
                fw = big.tile([128, N], f32, tag=f"fwd{dc}")
                fwd.append(fw)
            m_bc = big.tile([128, N], f32, tag="mask_bc")
            nc.gpsimd.partition_broadcast(m_bc[:], m_row[:])
            for dc in range(ND):
                nc.vector.tensor_tensor_scan(
                    out=fwd[dc][:], data0=m_bc[:], data1=repT[dc][:],
                    initial=0.0, op0=Alu.mult, op1=Alu.add)
            mp_bc = big.tile([128, N], f32, tag="mask_bc")
            nc.gpsimd.partition_broadcast(mp_bc[:], mp_row[:])
            for dc in range(ND):
                rv = revp.tile([128, N], f32, tag="rev")
                nc.vector.tensor_tensor_scan(
                    out=rv[:, ::-1], data0=mp_bc[:, ::-1],
                    data1=repT[dc][:, ::-1],
                    initial=0.0, op0=Alu.mult, op1=Alu.add)
                nc.vector.tensor_tensor(out=fwd[dc][:], in0=fwd[dc][:],
                                        in1=rv[:], op=Alu.add)
                nc.gpsimd.tensor_tensor(out=fwd[dc][:], in0=fwd[dc][:],
                                        in1=repT[dc][:], op=Alu.mult)

            # ---------- G = rep^T @ rep (fp32 exact) ----------
            G_all = big.tile([128, ND * D], f32, tag="G_all")
            for mc in range(ND):
                g_ps = pmm.tile([128, 512], f32, tag="mm")
                for t in range(NT):
                    nc.tensor.matmul(g_ps[:], rep[t][:, mc*128:(mc+1)*128],
                                     rep[t][:], start=(t == 0),
                                     stop=(t == NT-1))
                nc.scalar.copy(G_all[:, mc*D:(mc+1)*D], g_ps[:])

            # ---------- row norm^2 of rep (self term, ACT) ----------
            rnsq_cols = small.tile([128, NT], f32)
            for t in range(NT):
                sc = scrp.tile([128, D], f32, tag="scr_act")
                nc.scalar.activation(sc[:], rep[t][:], Act.Square,
                                     accum_out=rnsq_cols[:, t:t+1])

            # ---------- H = rep @ G ; ssq_i = <rep_i, H_i> ----------
            ssq_cols = small.tile([128, NT], f32)

            def h_tile(t):
                h_ps = pmm.tile([128, D], f32, tag="mm", name=f"h_ps{t}")
                for dc in range(ND):
                    nc.tensor.matmul(h_ps[:], repT[dc][:, t*128:(t+1)*128],
                                     G_all[:, dc*D:(dc+1)*D],
                                     start=(dc == 0), stop=(dc == ND-1))
                sc3 = scrp.tile([128, D], f32, tag="scr_stt", name=f"sc3_{t}")
                nc.vector.scalar_tensor_tensor(
                    out=sc3[:], in0=rep[t][:], scalar=0.0, in1=h_ps[:],
                    op0=Alu.add, op1=Alu.mult, accum_out=ssq_cols[:, t:t+1])

            for t in range(4):
                h_tile(t)

            # ---------- P partition-partial sums (gpsimd, in slack) ------
            nc.gpsimd.tensor_tensor(out=fwd[0][:], in0=fwd[0][:],
                                    in1=fwd[1][:], op=Alu.add)
            nc.gpsimd.tensor_tensor(out=fwd[2][:], in0=fwd[2][:],
                                    in1=fwd[3][:], op=Alu.add)
            nc.gpsimd.tensor_tensor(out=fwd[0][:], in0=fwd[0][:],
                                    in1=fwd[2][:], op=Alu.add)


            # ---------- u chain (interleaved with H stream) ----------
            u_cols = small.tile([128, ND], f32)
            for dc in range(ND):
                nc.vector.tensor_reduce(out=u_cols[:, dc:dc+1],
                                        in_=repT[dc][:],
                                        axis=mybir.AxisListType.X, op=Alu.add)
            ucT_ps = psm.tile([ND, 128], f32, tag="sm")
            nc.tensor.transpose(ucT_ps[:], u_cols[:], ident[:])
            ucT = small.tile([ND, 128], f32)
            nc.vector.tensor_copy(ucT[:], ucT_ps[:])
            u_row = small.tile([1, D], f32)
            nc.sync.dma_start(u_row[:], ucT[:])
            u_bc = small.tile([128, D], f32)
            nc.gpsimd.partition_broadcast(u_bc[:], u_row[:])

            rowsum_cols = small.tile([128, NT], f32)

            def rs_tile(t):
                sc4 = scrp.tile([128, D], f32, tag="scr_rs", name=f"sc4_{t}")
                nc.vector.scalar_tensor_tensor(
                    out=sc4[:], in0=rep[t][:], scalar=0.0, in1=u_bc[:],
                    op0=Alu.add, op1=Alu.mult,
                    accum_out=rowsum_cols[:, t:t+1])

            for t in range(4, 13):
                h_tile(t)
                rs_tile(t - 4)

            # ---------- q-row via PE ones-matvec over P_sum ----------
            q_row = rows.tile([1, N], f32, tag="rowbuf")
            for jc in range(NJ):
                qp = pmm.tile([1, 512], f32, tag="mm")
                nc.tensor.matmul(qp[:], ones_col[:],
                                 fwd[0][:, jc*512:(jc+1)*512],
                                 start=True, stop=True)
                nc.scalar.copy(q_row[:, jc*512:(jc+1)*512], qp[:])
            q_td = small.tile([16, 128], f32)
            nc.sync.dma_start(q_td[:], q_row.rearrange("a (p f) -> a p f", p=16))

            def to_16x128(cols, tag):
                ps = psm.tile([16, 128], f32, tag="sm", name=f"tps_{tag}")
                nc.tensor.transpose(ps[:], cols[:], ident[:])
                td = small.tile([16, 128], f32, name=f"td_{tag}")
                nc.vector.tensor_copy(td[:], ps[:])
                return td

            rnsq_td = to_16x128(rnsq_cols, "rnsq")
            q2 = small.tile([16, 128], f32)
            nc.vector.tensor_tensor(out=q2[:], in0=q_td[:], in1=rnsq_td[:],
                                    op=Alu.subtract)

            for t in range(13, NT):
                h_tile(t)
                rs_tile(t - 4)
            for t in range(NT - 4, NT):
                rs_tile(t)

            # ---------- ssq/rowsum -> (16,128) layout ----------
            ssq_td = to_16x128(ssq_cols, "ssq")
            rs_td = to_16x128(rowsum_cols, "rs")

            # ---------- per-row quantities in (16,128) ----------
            nrm = small.tile([16, 128], f32)
            nc.scalar.activation(nrm[:], ssq_td[:], Act.Sqrt)
            nc.vector.tensor_scalar(out=nrm[:], in0=nrm[:], scalar1=EPS_NORM,
                                    scalar2=None, op0=Alu.max)
            rnorm = small.tile([16, 128], f32)
            nc.vector.reciprocal(rnorm[:], nrm[:])
            deg = small.tile([16, 128], f32)
            nc.vector.tensor_tensor(out=deg[:], in0=rnorm[:], in1=rs_td[:],
                                    op=Alu.mult)
            w2 = small.tile([16, 128], f32)
            nc.vector.tensor_tensor(out=w2[:], in0=rnorm[:], in1=q2[:],
                                    op=Alu.mult)

            # ---------- segmented scans of deg/w2 with carries ----------
            segD0 = small.tile([16, 128], f32)
            nc.vector.tensor_tensor_scan(out=segD0[:], data0=m_td[:],
                                         data1=deg[:], initial=0.0,
                                         op0=Alu.mult, op1=Alu.add)
            segW0 = small.tile([16, 128], f32)
            nc.vector.tensor_tensor_scan(out=segW0[:], data0=m_td[:],
                                         data1=w2[:], initial=0.0,
                                         op0=Alu.mult, op1=Alu.add)

            def to_row(col_ap, tag):
                ps = psm.tile([1, 16], f32, tag="sm", name=f"tr_{tag}")
                nc.tensor.transpose(ps[:], col_ap, ident[:16, :16])
                row = small.tile([1, 16], f32, name=f"row_{tag}")
                nc.vector.tensor_copy(row[:], ps[:])
                return row

            aD_row = to_row(segD0[:, 127:128], "aD")
            aW_row = to_row(segW0[:, 127:128], "aW")

            def carry_col(a_row, tag):
                incl = small.tile([1, 16], f32, name=f"incl_{tag}")
                nc.vector.tensor_tensor_scan(out=incl[:], data0=bch_row[:],
                                             data1=a_row[:], initial=0.0,
                                             op0=Alu.mult, op1=Alu.add)
                excl = small.tile([1, 16], f32, name=f"excl_{tag}")
                nc.vector.memset(excl[:, 0:1], 0.0)
                nc.vector.tensor_copy(excl[:, 1:16], incl[:, 0:15])
                ps = psm.tile([16, 1], f32, tag="sm", name=f"cc_{tag}")
                nc.tensor.transpose(ps[:], excl[:], ident[:1, :1])
                col = small.tile([16, 1], f32, name=f"col_{tag}")
                nc.vector.tensor_copy(col[:], ps[:])
                return col

            iD_col = carry_col(aD_row, "D")
            iW_col = carry_col(aW_row, "W")

            segD = small.tile([16, 128], f32)
            nc.vector.scalar_tensor_tensor(
                out=segD[:], in0=Pm[:], scalar=iD_col[:], in1=segD0[:],
                op0=Alu.mult, op1=Alu.add)
            segW = small.tile([16, 128], f32)
            nc.vector.scalar_tensor_tensor(
                out=segW[:], in0=Pm[:], scalar=iW_col[:], in1=segW0[:],
                op0=Alu.mult, op1=Alu.add)

            # ---------- final reductions ----------
            Dl = small.tile([16, 128], f32)
            nc.vector.tensor_tensor(out=Dl[:], in0=segD[:], in1=l_td[:],
                                    op=Alu.mult)
            Wl = small.tile([16, 128], f32)
            nc.gpsimd.tensor_tensor(out=Wl[:], in0=segW[:], in1=l_td[:],
                                    op=Alu.mult)

            acc5 = small.tile([16, 5], f32)
            scr16 = small.tile([16, 128], f32)
            nc.vector.tensor_scalar(out=scr16[:], in0=Wl[:], scalar1=1.0,
                                    scalar2=0.0, op0=Alu.mult, op1=Alu.add,
                                    accum_out=acc5[:, 0:1])
            nc.vector.scalar_tensor_tensor(
                out=scr16[:], in0=Dl[:], scalar=0.0, in1=Dl[:],
                op0=Alu.add, op1=Alu.mult, accum_out=acc5[:, 1:2])
            num = small.tile([16, 128], f32)
            nc.vector.tensor_tensor(out=num[:], in0=Dl[:], in1=Wl[:],
                                    op=Alu.subtract)
            den = small.tile([16, 128], f32)
            nc.vector.tensor_tensor(out=den[:], in0=Dl[:], in1=Wl[:],
                                    op=Alu.add)
            lz = small.tile([16, 128], f32)
            nc.vector.tensor_scalar(out=lz[:], in0=l_td[:],
                                    scalar1=(EPS_COND - 1.0), scalar2=1.0,
                                    op0=Alu.mult, op1=Alu.add)
            nc.vector.tensor_tensor(out=den[:], in0=den[:], in1=lz[:],
                                    op=Alu.add)
            rden = small.tile([16, 128], f32)
            nc.vector.reciprocal(rden[:], den[:])
            nc.vector.scalar_tensor_tensor(
                out=scr16[:], in0=num[:], scalar=0.0, in1=rden[:],
                op0=Alu.add, op1=Alu.mult, accum_out=acc5[:, 2:3])
            nc.vector.tensor_scalar(out=scr16[:], in0=l_td[:], scalar1=1.0,
                                    scalar2=0.0, op0=Alu.mult, op1=Alu.add,
                                    accum_out=acc5[:, 3:4])
            nc.vector.tensor_scalar(out=scr16[:], in0=deg[:], scalar1=1.0,
                                    scalar2=0.0, op0=Alu.mult, op1=Alu.add,
                                    accum_out=acc5[:, 4:5])

            a5_ps = psm.tile([5, 16], f32, tag="sm")
            nc.tensor.transpose(a5_ps[:], acc5[:], ident[:16, :16])
            a5T = small.tile([5, 16], f32)
            nc.vector.tensor_copy(a5T[:], a5_ps[:])
            sums5 = small.tile([5, 1], f32)
            nc.vector.tensor_reduce(out=sums5[:], in_=a5T[:],
                                    axis=mybir.AxisListType.X, op=Alu.add)
            s5_ps = psm.tile([1, 5], f32, tag="sm")
            nc.tensor.transpose(s5_ps[:], sums5[:], ident[:5, :5])
            srow = small.tile([1, 5], f32)
            nc.vector.tensor_copy(srow[:], s5_ps[:])

            # srow = [W_sum, Dsq, cond_sum, n_comms, total]
            rtot = small.tile([1, 1], f32)
            nc.vector.reciprocal(rtot[:], srow[:, 4:5])
            t1 = small.tile([1, 1], f32)
            nc.vector.tensor_tensor(out=t1[:], in0=srow[:, 1:2], in1=rtot[:],
                                    op=Alu.mult)
            modn = small.tile([1, 1], f32)
            nc.vector.tensor_tensor(out=modn[:], in0=srow[:, 0:1], in1=t1[:],
                                    op=Alu.subtract)
            out_s = small.tile([1, 2], f32)
            nc.vector.tensor_tensor(out=out_s[:, 0:1], in0=modn[:],
                                    in1=rtot[:], op=Alu.mult)
            ncc = small.tile([1, 1], f32)
            nc.vector.tensor_scalar(out=ncc[:], in0=srow[:, 3:4], scalar1=1.0,
                                    scalar2=None, op0=Alu.max)
            rncc = small.tile([1, 1], f32)
            nc.vector.reciprocal(rncc[:], ncc[:])
            nc.vector.tensor_tensor(out=out_s[:, 1:2], in0=srow[:, 2:3],
                                    in1=rncc[:], op=Alu.mult)

            nc.sync.dma_start(out_d[:], out_s[:])

    nc.compile()
    return nc


def _get_compiled():
    global _COMPILED
    if _COMPILED is None:
        _COMPILED = _build()
    return _COMPILED


def _run(representations, boundaries, trace=False):
    from concourse.bass_utils import run_bass_kernel_spmd
    nc = _get_compiled()
    B = representations.shape[0]
    in_maps = [
        {"rep": np.ascontiguousarray(representations[i], dtype=np.float32),
         "bnd": np.ascontiguousarray(boundaries[i], dtype=np.int32)}
        for i in range(B)
    ]
    res = run_bass_kernel_spmd(nc, in_maps, list(range(B)), trace=trace)
    out = np.stack([res.results[i]["out"][0] for i in range(B)], axis=1)
    return out.astype(np.float32), res


def kernel(representations, boundaries):
    out, _ = _run(np.asarray(representations), np.asarray(boundaries))
    return out


# revision 30
# speedup vs baseline: 1.1333x; 1.1333x over previous
"""Trainium2 Bass kernel for nn_EpisodicMemory (modularity + conductance).

Per batch element (N=2048 rows, D=512 dims):
    S = rep @ rep.T            (never materialized!)
    S' = S / max(||S_row||, 1e-12)
    communities = contiguous runs given by cumsum(boundaries)
    mod  = (sum_same S' - sum_c D_c^2/total) / total
    cond = mean_c (D_c - W_c)/(W_c + D_c + 1e-10)

S-free formulation (everything is exact-fp32-grade):
    G = rep^T rep (512x512 Gram);  H = rep @ G;  ssq_i = <rep_i, H_i>
    rowsum_i = <rep_i, u>,  u = sum_j rep_j
    q_i = sum_{j in comm(i)} S_ij = <rep_i, R(c_i)> via forward+reverse
          segmented scans over rep^T plus a ones-matvec partition reduce
          (minus the double-counted self term ||rep_i||^2).
    W_c, D_c from segmented scans of rnorm*q and rnorm*rowsum in a
    (16,128) layout with cross-partition carry fix-up.

Sharding: data-parallel over the batch axis, one batch element per core,
8 NeuronCores. Full inputs in, full (2, 8) output out.
"""
import sys
if '/opt/trn_rl_repo' not in sys.path:
    sys.path.insert(0, '/opt/trn_rl_repo')

import numpy as np

N = 2048
D = 512
NT = N // 128          # 16 row tiles of rep
ND = D // 128          # 4 partition chunks of repT
NJ = N // 512          # 4 free chunks of 512
EPS_NORM = 1e-12
EPS_COND = 1e-10

_COMPILED = None


def _build():
    import concourse.bacc as bacc
    import concourse.tile as tile
    from concourse import mybir
    from concourse.masks import make_identity

    f32 = mybir.dt.float32
    f32r = mybir.dt.float32r
    i32 = mybir.dt.int32
    Alu = mybir.AluOpType
    Act = mybir.ActivationFunctionType

    nc = bacc.Bacc("TRN2", target_bir_lowering=False, debug=False)
    rep_d = nc.dram_tensor("rep", [N, D], f32, kind="ExternalInput")
    bnd_d = nc.dram_tensor("bnd", [N], i32, kind="ExternalInput")
    out_d = nc.dram_tensor("out", [1, 2], f32, kind="ExternalOutput")

    rep_tiles_d = rep_d.rearrange("(t p) d -> t p d", p=128)
    b_row_d = bnd_d.rearrange("(a f) -> a f", a=1)

    with tile.TileContext(nc) as tc:
        with (
            tc.tile_pool(name="big", bufs=1) as big,
            tc.tile_pool(name="small", bufs=1) as small,
            tc.tile_pool(name="scr", bufs=4) as scrp,
            tc.tile_pool(name="rows", bufs=3) as rows,
            tc.tile_pool(name="revp", bufs=2) as revp,
            tc.tile_pool(name="pmm", bufs=6, space="PSUM") as pmm,
            tc.tile_pool(name="psm", bufs=2, space="PSUM") as psm,
        ):
            # ---------- constants ----------
            ident = small.tile([128, 128], f32)
            make_identity(nc, ident[:])
            ones_col = small.tile([128, 1], f32)
            nc.vector.memset(ones_col[:], 1.0)

            # ---------- load inputs (issue split over 2 HWDGE queues) ----
            b_row = small.tile([1, N], i32)
            nc.sync.dma_start(b_row[:], b_row_d[:])
            rep = []
            for t in range(NT):
                rt = big.tile([128, D], f32, tag=f"rep{t}")
                eng = nc.sync if t % 2 == 0 else nc.scalar
                eng.dma_start(rt[:], rep_tiles_d[t])
                rep.append(rt)

            # ---------- masks ----------
            bf_row = rows.tile([1, N], f32, tag="rowbuf")
            nc.scalar.activation(bf_row[:], b_row[:], Act.Copy)
            m_row = rows.tile([1, N], f32, tag="rowbuf")   # 0 at starts
            nc.scalar.activation(m_row[:], bf_row[:], Act.Copy,
                                 bias=1.0, scale=-1.0)
            l_row = rows.tile([1, N], f32, tag="rowbuf")   # 1 at ends
            nc.vector.memset(l_row[:, N-1:N], 1.0)
            nc.scalar.activation(l_row[:, 0:N-1], bf_row[:, 1:N], Act.Copy)
            mp_row = rows.tile([1, N], f32, tag="rowbuf")  # 0 at ends
            nc.scalar.activation(mp_row[:], l_row[:], Act.Copy,
                                 bias=1.0, scale=-1.0)

            m_td = small.tile([16, 128], f32)
            nc.sync.dma_start(m_td[:], m_row.rearrange("a (p f) -> a p f", p=16))
            l_td = small.tile([16, 128], f32)
            nc.sync.dma_start(l_td[:], l_row.rearrange("a (p f) -> a p f", p=16))

            Pm = small.tile([16, 128], f32)
            nc.vector.tensor_tensor_scan(out=Pm[:], data0=m_td[:],
                                         data1=m_td[:], initial=1.0,
                                         op0=Alu.mult, op1=Alu.bypass)
            bch_ps = psm.tile([1, 16], f32, tag="sm")
            nc.tensor.transpose(bch_ps[:], Pm[:, 127:128], ident[:16, :16])
            bch_row = small.tile([1, 16], f32)
            nc.vector.tensor_copy(bch_row[:], bch_ps[:])

            # ---------- transpose rep -> repT (PE; PSUM->SBUF on ACT) ----
            repT = []
            for dc in range(ND):
                rT = big.tile([128, N], f32, tag=f"repT{dc}")
                repT.append(rT)
            for dc in range(ND):
                for tg in range(4):
                    tp_ps = pmm.tile([128, 512], f32, tag="mm")
                    for tt in range(4):
                        t = tg * 4 + tt
                        nc.tensor.transpose(
                            tp_ps[:, tt*128:(tt+1)*128],
                            rep[t][:, dc*128:(dc+1)*128], ident[:])
                    nc.scalar.copy(repT[dc][:, tg*512:(tg+1)*512], tp_ps[:])

            # ---------- segmented scans over repT (DVE) + P (gpsimd) ----
            fwd = []
            for dc in range(ND):
                fw = big.tile([128, N], f32, tag=f"fwd{dc}")
                fwd.append(fw)
            m_bc = big.tile([128, N], f32, tag="mask_bc")
            nc.gpsimd.partition_broadcast(m_bc[:], m_row[:])
            for dc in range(ND):
                nc.vector.tensor_tensor_scan(
                    out=fwd[dc][:], data0=m_bc[:], data1=repT[dc][:],
                    initial=0.0, op0=Alu.mult, op1=Alu.add)
            mp_bc = big.tile([128, N], f32, tag="mask_bc")
            nc.gpsimd.partition_broadcast(mp_bc[:], mp_row[:])
            for dc in range(ND):
                rv = revp.tile([128, N], f32, tag="rev")
                nc.vector.tensor_tensor_scan(
                    out=rv[:, ::-1], data0=mp_bc[:, ::-1],
                    data1=repT[dc][:, ::-1],
                    initial=0.0, op0=Alu.mult, op1=Alu.add)
                nc.vector.tensor_tensor(out=fwd[dc][:], in0=fwd[dc][:],
                                        in1=rv[:], op=Alu.add)
                nc.gpsimd.tensor_tensor(out=fwd[dc][:], in0=fwd[dc][:],
                                        in1=repT[dc][:], op=Alu.mult)

            # ---------- G = rep^T @ rep (fp32 exact) ----------
            G_all = big.tile([128, ND * D], f32, tag="G_all")
            for mc in range(ND):
                g_ps = pmm.tile([128, 512], f32, tag="mm")
                for t in range(NT):
                    nc.tensor.matmul(g_ps[:], rep[t][:, mc*128:(mc+1)*128],
                                     rep[t][:], start=(t == 0),
                                     stop=(t == NT-1))
                nc.scalar.copy(G_all[:, mc*D:(mc+1)*D], g_ps[:])

            # ---------- row norm^2 of rep (self term, ACT) ----------
            rnsq_cols = small.tile([128, NT], f32)
            for t in range(NT):
                sc = scrp.tile([128, D], f32, tag="scr_act")
                nc.scalar.activation(sc[:], rep[t][:], Act.Square,
                                     accum_out=rnsq_cols[:, t:t+1])

            # ---------- H = rep @ G ; ssq_i = <rep_i, H_i> ----------
            ssq_cols = small.tile([128, NT], f32)

            def h_tile(t):
                h_ps = pmm.tile([128, D], f32, tag="mm", name=f"h_ps{t}")
                for dc in range(ND):
                    nc.tensor.matmul(h_ps[:], repT[dc][:, t*128:(t+1)*128],
                                     G_all[:, dc*D:(dc+1)*D],
                                     start=(dc == 0), stop=(dc == ND-1))
                sc3 = scrp.tile([128, D], f32, tag="scr_stt", name=f"sc3_{t}")
                nc.vector.scalar_tensor_tensor(
                    out=sc3[:], in0=rep[t][:], scalar=0.0, in1=h_ps[:],
                    op0=Alu.add, op1=Alu.mult, accum_out=ssq_cols[:, t:t+1])

            for t in range(4):
                h_tile(t)

            # ---------- P partition-partial sums (gpsimd, in slack) ------
            nc.gpsimd.tensor_tensor(out=fwd[0][:], in0=fwd[0][:],
                                    in1=fwd[1][:], op=Alu.add)
            nc.gpsimd.tensor_tensor(out=fwd[2][:], in0=fwd[2][:],
                                    in1=fwd[3][:], op=Alu.add)
            nc.gpsimd.tensor_tensor(out=fwd[0][:], in0=fwd[0][:],
                                    in1=fwd[2][:], op=Alu.add)


            # ---------- u chain (interleaved with H stream) ----------
            u_cols = small.tile([128, ND], f32)
            for dc in range(ND):
                nc.vector.tensor_reduce(out=u_cols[:, dc:dc+1],
                                        in_=repT[dc][:],
                                        axis=mybir.AxisListType.X, op=Alu.add)
            ucT_ps = psm.tile([ND, 128], f32, tag="sm")
            nc.tensor.transpose(ucT_ps[:], u_cols[:], ident[:])
            ucT = small.tile([ND, 128], f32)
            nc.vector.tensor_copy(ucT[:], ucT_ps[:])
            u_row = small.tile([1, D], f32)
            nc.sync.dma_start(u_row[:], ucT[:])
            u_bc = small.tile([128, D], f32)
            nc.gpsimd.partition_broadcast(u_bc[:], u_row[:])

            rowsum_cols = small.tile([128, NT], f32)

            def rs_tile(t):
                sc4 = scrp.tile([128, D], f32, tag="scr_rs", name=f"sc4_{t}")
                nc.vector.scalar_tensor_tensor(
                    out=sc4[:], in0=rep[t][:], scalar=0.0, in1=u_bc[:],
                    op0=Alu.add, op1=Alu.mult,
                    accum_out=rowsum_cols[:, t:t+1])

            for t in range(4, 12):
                h_tile(t)
                rs_tile(t - 4)

            # ---------- q-row via PE ones-matvec over P_sum ----------
            q_row = rows.tile([1, N], f32, tag="rowbuf")
            for jc in range(NJ):
                qp = pmm.tile([1, 512], f32, tag="mm")
                nc.tensor.matmul(qp[:], ones_col[:],
                                 fwd[0][:, jc*512:(jc+1)*512],
                                 start=True, stop=True)
                nc.scalar.copy(q_row[:, jc*512:(jc+1)*512], qp[:])
            q_td = small.tile([16, 128], f32)
            nc.sync.dma_start(q_td[:], q_row.rearrange("a (p f) -> a p f", p=16))

            def to_16x128(cols, tag):
                ps = psm.tile([16, 128], f32, tag="sm", name=f"tps_{tag}")
                nc.tensor.transpose(ps[:], cols[:], ident[:])
                td = small.tile([16, 128], f32, name=f"td_{tag}")
                nc.vector.tensor_copy(td[:], ps[:])
                return td

            rnsq_td = to_16x128(rnsq_cols, "rnsq")
            q2 = small.tile([16, 128], f32)
            nc.vector.tensor_tensor(out=q2[:], in0=q_td[:], in1=rnsq_td[:],
                                    op=Alu.subtract)

            for t in range(12, NT):
                h_tile(t)
                rs_tile(t - 4)
            for t in range(NT - 4, NT):
                rs_tile(t)

            # ---------- ssq/rowsum -> (16,128) layout ----------
            ssq_td = to_16x128(ssq_cols, "ssq")
            rs_td = to_16x128(rowsum_cols, "rs")

            # ---------- per-row quantities in (16,128) ----------
            nrm = small.tile([16, 128], f32)
            nc.scalar.activation(nrm[:], ssq_td[:], Act.Sqrt)
            nc.vector.tensor_scalar(out=nrm[:], in0=nrm[:], scalar1=EPS_NORM,
                                    scalar2=None, op0=Alu.max)
            rnorm = small.tile([16, 128], f32)
            nc.vector.reciprocal(rnorm[:], nrm[:])
            deg = small.tile([16, 128], f32)
            nc.vector.tensor_tensor(out=deg[:], in0=rnorm[:], in1=rs_td[:],
                                    op=Alu.mult)
            w2 = small.tile([16, 128], f32)
            nc.vector.tensor_tensor(out=w2[:], in0=rnorm[:], in1=q2[:],
                                    op=Alu.mult)

            # ---------- segmented scans of deg/w2 with carries ----------
            segD0 = small.tile([16, 128], f32)
            nc.vector.tensor_tensor_scan(out=segD0[:], data0=m_td[:],
                                         data1=deg[:], initial=0.0,
                                         op0=Alu.mult, op1=Alu.add)
            segW0 = small.tile([16, 128], f32)
            nc.vector.tensor_tensor_scan(out=segW0[:], data0=m_td[:],
                                         data1=w2[:], initial=0.0,
                                         op0=Alu.mult, op1=Alu.add)

            def to_row(col_ap, tag):
                ps = psm.tile([1, 16], f32, tag="sm", name=f"tr_{tag}")
                nc.tensor.transpose(ps[:], col_ap, ident[:16, :16])
                row = small.tile([1, 16], f32, name=f"row_{tag}")
                nc.vector.tensor_copy(row[:], ps[:])
                return row

            aD_row = to_row(segD0[:, 127:128], "aD")
            aW_row = to_row(segW0[:, 127:128], "aW")

            def carry_col(a_row, tag):
                incl = small.tile([1, 16], f32, name=f"incl_{tag}")
                nc.vector.tensor_tensor_scan(out=incl[:], data0=bch_row[:],
                                             data1=a_row[:], initial=0.0,
                                             op0=Alu.mult, op1=Alu.add)
                excl = small.tile([1, 16], f32, name=f"excl_{tag}")
                nc.vector.memset(excl[:, 0:1], 0.0)
                nc.vector.tensor_copy(excl[:, 1:16], incl[:, 0:15])
                ps = psm.tile([16, 1], f32, tag="sm", name=f"cc_{tag}")
                nc.tensor.transpose(ps[:], excl[:], ident[:1, :1])
                col = small.tile([16, 1], f32, name=f"col_{tag}")
                nc.vector.tensor_copy(col[:], ps[:])
                return col

            iD_col = carry_col(aD_row, "D")
            iW_col = carry_col(aW_row, "W")

            segD = small.tile([16, 128], f32)
            nc.vector.scalar_tensor_tensor(
                out=segD[:], in0=Pm[:], scalar=iD_col[:], in1=segD0[:],
                op0=Alu.mult, op1=Alu.add)
            segW = small.tile([16, 128], f32)
            nc.vector.scalar_tensor_tensor(
                out=segW[:], in0=Pm[:], scalar=iW_col[:], in1=segW0[:],
                op0=Alu.mult, op1=Alu.add)

            # ---------- final reductions ----------
            Dl = small.tile([16, 128], f32)
            nc.vector.tensor_tensor(out=Dl[:], in0=segD[:], in1=l_td[:],
                                    op=Alu.mult)
            Wl = small.tile([16, 128], f32)
            nc.gpsimd.tensor_tensor(out=Wl[:], in0=segW[:], in1=l_td[:],
                                    op=Alu.mult)

            acc5 = small.tile([16, 5], f32)
            scr16 = small.tile([16, 128], f32)
            nc.vector.tensor_scalar(out=scr16[:], in0=Wl[:], scalar1=1.0,
                                    scalar2=0.0, op0=Alu.mult, op1=Alu.add,
                                    accum_out=acc5[:, 0:1])
            nc.vector.scalar_tensor_tensor(
                out=scr16[:], in0=Dl[:], scalar=0.0, in1=Dl[:],
                op0=Alu.add, op1=Alu.mult, accum_out=acc5[:, 1:2])
            num = small.tile([16, 128], f32)
            nc.vector.tensor_tensor(out=num[:], in0=Dl[:], in1=Wl[:],
                                    op=Alu.subtract)
            den = small.tile([16, 128], f32)
            nc.vector.tensor_tensor(out=den[:], in0=Dl[:], in1=Wl[:],
                                    op=Alu.add)
            lz = small.tile([16, 128], f32)
            nc.vector.tensor_scalar(out=lz[:], in0=l_td[:],
                                    scalar1=(EPS_COND - 1.0), scalar2=1.0,
                                    op0=Alu.mult, op1=Alu.add)
            nc.vector.tensor_tensor(out=den[:], in0=den[:], in1=lz[:],
                                    op=Alu.add)
            rden = small.tile([16, 128], f32)
            nc.vector.reciprocal(rden[:], den[:])
            nc.vector.scalar_tensor_tensor(
                out=scr16[:], in0=num[:], scalar=0.0, in1=rden[:],
                op0=Alu.add, op1=Alu.mult, accum_out=acc5[:, 2:3])
            nc.vector.tensor_scalar(out=scr16[:], in0=l_td[:], scalar1=1.0,
                                    scalar2=0.0, op0=Alu.mult, op1=Alu.add,
                                    accum_out=acc5[:, 3:4])
            nc.vector.tensor_scalar(out=scr16[:], in0=deg[:], scalar1=1.0,
                                    scalar2=0.0, op0=Alu.mult, op1=Alu.add,
                                    accum_out=acc5[:, 4:5])

            a5_ps = psm.tile([5, 16], f32, tag="sm")
            nc.tensor.transpose(a5_ps[:], acc5[:], ident[:16, :16])
            a5T = small.tile([5, 16], f32)
            nc.vector.tensor_copy(a5T[:], a5_ps[:])
            sums5 = small.tile([5, 1], f32)
            nc.vector.tensor_reduce(out=sums5[:], in_=a5T[:],
                                    axis=mybir.AxisListType.X, op=Alu.add)
            s5_ps = psm.tile([1, 5], f32, tag="sm")
            nc.tensor.transpose(s5_ps[:], sums5[:], ident[:5, :5])
            srow = small.tile([1, 5], f32)
            nc.vector.tensor_copy(srow[:], s5_ps[:])

            # srow = [W_sum, Dsq, cond_sum, n_comms, total]
            rtot = small.tile([1, 1], f32)
            nc.vector.reciprocal(rtot[:], srow[:, 4:5])
            t1 = small.tile([1, 1], f32)
            nc.vector.tensor_tensor(out=t1[:], in0=srow[:, 1:2], in1=rtot[:],
                                    op=Alu.mult)
            modn = small.tile([1, 1], f32)
            nc.vector.tensor_tensor(out=modn[:], in0=srow[:, 0:1], in1=t1[:],
                                    op=Alu.subtract)
            out_s = small.tile([1, 2], f32)
            nc.vector.tensor_tensor(out=out_s[:, 0:1], in0=modn[:],
                                    in1=rtot[:], op=Alu.mult)
            ncc = small.tile([1, 1], f32)
            nc.vector.tensor_scalar(out=ncc[:], in0=srow[:, 3:4], scalar1=1.0,
                                    scalar2=None, op0=Alu.max)
            rncc = small.tile([1, 1], f32)
            nc.vector.reciprocal(rncc[:], ncc[:])
            nc.vector.tensor_tensor(out=out_s[:, 1:2], in0=srow[:, 2:3],
                                    in1=rncc[:], op=Alu.mult)

            nc.sync.dma_start(out_d[:], out_s[:])

    nc.compile()
    return nc


def _get_compiled():
    global _COMPILED
    if _COMPILED is None:
        _COMPILED = _build()
    return _COMPILED


def _run(representations, boundaries, trace=False):
    from concourse.bass_utils import run_bass_kernel_spmd
    nc = _get_compiled()
    B = representations.shape[0]
    in_maps = [
        {"rep": np.ascontiguousarray(representations[i], dtype=np.float32),
         "bnd": np.ascontiguousarray(boundaries[i], dtype=np.int32)}
        for i in range(B)
    ]
    res = run_bass_kernel_spmd(nc, in_maps, list(range(B)), trace=trace)
    out = np.stack([res.results[i]["out"][0] for i in range(B)], axis=1)
    return out.astype(np.float32), res


def kernel(representations, boundaries):
    out, _ = _run(np.asarray(representations), np.asarray(boundaries))
    return out


# revision 32
# speedup vs baseline: 1.1363x; 1.0027x over previous
"""Trainium2 Bass kernel for nn_EpisodicMemory (modularity + conductance).

Per batch element (N=2048 rows, D=512 dims):
    S = rep @ rep.T            (never materialized!)
    S' = S / max(||S_row||, 1e-12)
    communities = contiguous runs given by cumsum(boundaries)
    mod  = (sum_same S' - sum_c D_c^2/total) / total
    cond = mean_c (D_c - W_c)/(W_c + D_c + 1e-10)

S-free formulation (everything is exact-fp32-grade):
    G = rep^T rep (512x512 Gram);  H = rep @ G;  ssq_i = <rep_i, H_i>
    rowsum_i = <rep_i, u>,  u = sum_j rep_j
    q_i = sum_{j in comm(i)} S_ij = <rep_i, R(c_i)> via forward+reverse
          segmented scans over rep^T plus a ones-matvec partition reduce
          (minus the double-counted self term ||rep_i||^2).
    W_c, D_c from segmented scans of rnorm*q and rnorm*rowsum in a
    (16,128) layout with cross-partition carry fix-up.

Sharding: data-parallel over the batch axis, one batch element per core,
8 NeuronCores. Full inputs in, full (2, 8) output out.
"""
import sys
if '/opt/trn_rl_repo' not in sys.path:
    sys.path.insert(0, '/opt/trn_rl_repo')

import numpy as np

N = 2048
D = 512
NT = N // 128          # 16 row tiles of rep
ND = D // 128          # 4 partition chunks of repT
NJ = N // 512          # 4 free chunks of 512
EPS_NORM = 1e-12
EPS_COND = 1e-10

_COMPILED = None


def _build():
    import concourse.bacc as bacc
    import concourse.tile as tile
    from concourse import mybir
    from concourse.masks import make_identity

    f32 = mybir.dt.float32
    f32r = mybir.dt.float32r
    i32 = mybir.dt.int32
    Alu = mybir.AluOpType
    Act = mybir.ActivationFunctionType

    nc = bacc.Bacc("TRN2", target_bir_lowering=False, debug=False)
    rep_d = nc.dram_tensor("rep", [N, D], f32, kind="ExternalInput")
    bnd_d = nc.dram_tensor("bnd", [N], i32, kind="ExternalInput")
    out_d = nc.dram_tensor("out", [1, 2], f32, kind="ExternalOutput")

    rep_tiles_d = rep_d.rearrange("(t p) d -> t p d", p=128)
    b_row_d = bnd_d.rearrange("(a f) -> a f", a=1)

    with tile.TileContext(nc) as tc:
        with (
            tc.tile_pool(name="big", bufs=1) as big,
            tc.tile_pool(name="small", bufs=1) as small,
            tc.tile_pool(name="scr", bufs=4) as scrp,
            tc.tile_pool(name="rows", bufs=3) as rows,
            tc.tile_pool(name="revp", bufs=2) as revp,
            tc.tile_pool(name="pmm", bufs=6, space="PSUM") as pmm,
            tc.tile_pool(name="psm", bufs=2, space="PSUM") as psm,
        ):
            # ---------- constants ----------
            ident = small.tile([128, 128], f32)
            make_identity(nc, ident[:])
            ones_col = small.tile([128, 1], f32)
            nc.vector.memset(ones_col[:], 1.0)

            # ---------- load inputs (issue split over 2 HWDGE queues) ----
            b_row = small.tile([1, N], i32)
            nc.sync.dma_start(b_row[:], b_row_d[:])
            rep = []
            for t in range(NT):
                rt = big.tile([128, D], f32, tag=f"rep{t}")
                eng = nc.sync if t % 2 == 0 else nc.scalar
                eng.dma_start(rt[:], rep_tiles_d[t])
                rep.append(rt)

            # ---------- masks ----------
            bf_row = rows.tile([1, N], f32, tag="rowbuf")
            nc.scalar.activation(bf_row[:], b_row[:], Act.Copy)
            m_row = rows.tile([1, N], f32, tag="rowbuf")   # 0 at starts
            nc.scalar.activation(m_row[:], bf_row[:], Act.Copy,
                                 bias=1.0, scale=-1.0)
            l_row = rows.tile([1, N], f32, tag="rowbuf")   # 1 at ends
            nc.vector.memset(l_row[:, N-1:N], 1.0)
            nc.scalar.activation(l_row[:, 0:N-1], bf_row[:, 1:N], Act.Copy)
            mp_row = rows.tile([1, N], f32, tag="rowbuf")  # 0 at ends
            nc.scalar.activation(mp_row[:], l_row[:], Act.Copy,
                                 bias=1.0, scale=-1.0)

            m_td = small.tile([16, 128], f32)
            nc.sync.dma_start(m_td[:], m_row.rearrange("a (p f) -> a p f", p=16))
            l_td = small.tile([16, 128], f32)
            nc.sync.dma_start(l_td[:], l_row.rearrange("a (p f) -> a p f", p=16))

            Pm = small.tile([16, 128], f32)
            nc.vector.tensor_tensor_scan(out=Pm[:], data0=m_td[:],
                                         data1=m_td[:], initial=1.0,
                                         op0=Alu.mult, op1=Alu.bypass)
            bch_ps = psm.tile([1, 16], f32, tag="sm")
            nc.tensor.transpose(bch_ps[:], Pm[:, 127:128], ident[:16, :16])
            bch_row = small.tile([1, 16], f32)
            nc.vector.tensor_copy(bch_row[:], bch_ps[:])

            # ---------- transpose rep -> repT (PE; PSUM->SBUF on ACT) ----
            repT = []
            for dc in range(ND):
                rT = big.tile([128, N], f32, tag=f"repT{dc}")
                repT.append(rT)
            for dc in range(ND):
                for tg in range(4):
                    tp_ps = pmm.tile([128, 512], f32, tag="mm")
                    for tt in range(4):
                        t = tg * 4 + tt
                        nc.tensor.transpose(
                            tp_ps[:, tt*128:(tt+1)*128],
                            rep[t][:, dc*128:(dc+1)*128], ident[:])
                    nc.scalar.copy(repT[dc][:, tg*512:(tg+1)*512], tp_ps[:])

            # ---------- segmented scans over repT (DVE) + P (gpsimd) ----
            fwd = []
            for dc in range(ND):
                fw = big.tile([128, N], f32, tag=f"fwd{dc}")
                fwd.append(fw)
            m_bc = big.tile([128, N], f32, tag="mask_bc")
            nc.gpsimd.partition_broadcast(m_bc[:], m_row[:])
            for dc in range(ND):
                nc.vector.tensor_tensor_scan(
                    out=fwd[dc][:], data0=m_bc[:], data1=repT[dc][:],
                    initial=0.0, op0=Alu.mult, op1=Alu.add)
            mp_bc = big.tile([128, N], f32, tag="mask_bc")
            nc.gpsimd.partition_broadcast(mp_bc[:], mp_row[:])
            for dc in range(ND):
                rv = revp.tile([128, N], f32, tag="rev")
                nc.vector.tensor_tensor_scan(
                    out=rv[:, ::-1], data0=mp_bc[:, ::-1],
                    data1=repT[dc][:, ::-1],
                    initial=0.0, op0=Alu.mult, op1=Alu.add)
                nc.vector.tensor_tensor(out=fwd[dc][:], in0=fwd[dc][:],
                                        in1=rv[:], op=Alu.add)
                nc.gpsimd.tensor_tensor(out=fwd[dc][:], in0=fwd[dc][:],
                                        in1=repT[dc][:], op=Alu.mult)

            # ---------- G = rep^T @ rep (fp32 exact) ----------
            G_all = big.tile([128, ND * D], f32, tag="G_all")
            for mc in range(ND):
                g_ps = pmm.tile([128, 512], f32, tag="mm")
                for t in range(NT):
                    nc.tensor.matmul(g_ps[:], rep[t][:, mc*128:(mc+1)*128],
                                     rep[t][:], start=(t == 0),
                                     stop=(t == NT-1))
                nc.scalar.copy(G_all[:, mc*D:(mc+1)*D], g_ps[:])

            # ---------- row norm^2 of rep (self term, ACT) ----------
            rnsq_cols = small.tile([128, NT], f32)
            for t in range(NT):
                sc = scrp.tile([128, D], f32, tag="scr_act")
                nc.scalar.activation(sc[:], rep[t][:], Act.Square,
                                     accum_out=rnsq_cols[:, t:t+1])

            # ---------- H = rep @ G ; ssq_i = <rep_i, H_i> ----------
            ssq_cols = small.tile([128, NT], f32)

            def h_tile(t):
                h_ps = pmm.tile([128, D], f32, tag="mm", name=f"h_ps{t}")
                for dc in range(ND):
                    nc.tensor.matmul(h_ps[:], repT[dc][:, t*128:(t+1)*128],
                                     G_all[:, dc*D:(dc+1)*D],
                                     start=(dc == 0), stop=(dc == ND-1))
                sc3 = scrp.tile([128, D], f32, tag="scr_stt", name=f"sc3_{t}")
                nc.vector.scalar_tensor_tensor(
                    out=sc3[:], in0=rep[t][:], scalar=0.0, in1=h_ps[:],
                    op0=Alu.add, op1=Alu.mult, accum_out=ssq_cols[:, t:t+1])

            for t in range(4):
                h_tile(t)

            # ---------- P partition-partial sums (gpsimd, in slack) ------
            nc.gpsimd.tensor_tensor(out=fwd[0][:], in0=fwd[0][:],
                                    in1=fwd[1][:], op=Alu.add)
            nc.gpsimd.tensor_tensor(out=fwd[2][:], in0=fwd[2][:],
                                    in1=fwd[3][:], op=Alu.add)
            nc.gpsimd.tensor_tensor(out=fwd[0][:], in0=fwd[0][:],
                                    in1=fwd[2][:], op=Alu.add)


            # ---------- u chain (interleaved with H stream) ----------
            u_cols = small.tile([128, ND], f32)
            for dc in range(ND):
                nc.vector.tensor_reduce(out=u_cols[:, dc:dc+1],
                                        in_=repT[dc][:],
                                        axis=mybir.AxisListType.X, op=Alu.add)
            ucT_ps = psm.tile([ND, 128], f32, tag="sm")
            nc.tensor.transpose(ucT_ps[:], u_cols[:], ident[:])
            ucT = small.tile([ND, 128], f32)
            nc.vector.tensor_copy(ucT[:], ucT_ps[:])
            u_row = small.tile([1, D], f32)
            nc.sync.dma_start(u_row[:], ucT[:])
            u_bc = small.tile([128, D], f32)
            nc.gpsimd.partition_broadcast(u_bc[:], u_row[:])

            rowsum_cols = small.tile([128, NT], f32)

            def rs_tile(t):
                sc4 = scrp.tile([128, D], f32, tag="scr_rs", name=f"sc4_{t}")
                nc.vector.scalar_tensor_tensor(
                    out=sc4[:], in0=rep[t][:], scalar=0.0, in1=u_bc[:],
                    op0=Alu.add, op1=Alu.mult,
                    accum_out=rowsum_cols[:, t:t+1])

            for t in range(4, 12):
                h_tile(t)
                rs_tile(t - 4)

            # ---------- q-row via PE ones-matvec over P_sum ----------
            q_row = rows.tile([1, N], f32, tag="rowbuf")
            for jc in range(NJ):
                qp = pmm.tile([1, 512], f32, tag="mm")
                nc.tensor.matmul(qp[:], ones_col[:],
                                 fwd[0][:, jc*512:(jc+1)*512],
                                 start=True, stop=True)
                nc.scalar.copy(q_row[:, jc*512:(jc+1)*512], qp[:])
            q_td = small.tile([16, 128], f32)
            nc.sync.dma_start(q_td[:], q_row.rearrange("a (p f) -> a p f", p=16))

            def to_16x128(cols, tag):
                ps = psm.tile([16, 128], f32, tag="sm", name=f"tps_{tag}")
                nc.tensor.transpose(ps[:], cols[:], ident[:])
                td = small.tile([16, 128], f32, name=f"td_{tag}")
                nc.vector.tensor_copy(td[:], ps[:])
                return td

            rnsq_td = to_16x128(rnsq_cols, "rnsq")
            q2 = small.tile([16, 128], f32)
            nc.vector.tensor_tensor(out=q2[:], in0=q_td[:], in1=rnsq_td[:],
                                    op=Alu.subtract)

            for t in range(12, NT):
                h_tile(t)
                rs_tile(t - 4)
            for t in range(NT - 4, NT):
                rs_tile(t)

            # ---------- ssq/rowsum -> (16,128) layout ----------
            ssq_td = to_16x128(ssq_cols, "ssq")
            rs_td = to_16x128(rowsum_cols, "rs")

            # ---------- per-row quantities in (16,128) ----------
            nrm = small.tile([16, 128], f32)
            nc.scalar.activation(nrm[:], ssq_td[:], Act.Sqrt)
            nc.vector.tensor_scalar(out=nrm[:], in0=nrm[:], scalar1=EPS_NORM,
                                    scalar2=None, op0=Alu.max)
            rnorm = small.tile([16, 128], f32)
            nc.vector.reciprocal(rnorm[:], nrm[:])
            deg = small.tile([16, 128], f32)
            nc.vector.tensor_tensor(out=deg[:], in0=rnorm[:], in1=rs_td[:],
                                    op=Alu.mult)
            w2 = small.tile([16, 128], f32)
            nc.vector.tensor_tensor(out=w2[:], in0=rnorm[:], in1=q2[:],
                                    op=Alu.mult)

            # ---------- segmented scans of deg/w2 with carries ----------
            segD0 = small.tile([16, 128], f32)
            nc.vector.tensor_tensor_scan(out=segD0[:], data0=m_td[:],
                                         data1=deg[:], initial=0.0,
                                         op0=Alu.mult, op1=Alu.add)
            segW0 = small.tile([16, 128], f32)
            nc.vector.tensor_tensor_scan(out=segW0[:], data0=m_td[:],
                                         data1=w2[:], initial=0.0,
                                         op0=Alu.mult, op1=Alu.add)

            def to_row(col_ap, tag):
                ps = psm.tile([1, 16], f32, tag="sm", name=f"tr_{tag}")
                nc.tensor.transpose(ps[:], col_ap, ident[:16, :16])
                row = small.tile([1, 16], f32, name=f"row_{tag}")
                nc.vector.tensor_copy(row[:], ps[:])
                return row

            aD_row = to_row(segD0[:, 127:128], "aD")
            aW_row = to_row(segW0[:, 127:128], "aW")

            def carry_col(a_row, tag):
                incl = small.tile([1, 16], f32, name=f"incl_{tag}")
                nc.vector.tensor_tensor_scan(out=incl[:], data0=bch_row[:],
                                             data1=a_row[:], initial=0.0,
                                             op0=Alu.mult, op1=Alu.add)
                excl = small.tile([1, 16], f32, name=f"excl_{tag}")
                nc.vector.memset(excl[:, 0:1], 0.0)
                nc.vector.tensor_copy(excl[:, 1:16], incl[:, 0:15])
                ps = psm.tile([16, 1], f32, tag="sm", name=f"cc_{tag}")
                nc.tensor.transpose(ps[:], excl[:], ident[:1, :1])
                col = small.tile([16, 1], f32, name=f"col_{tag}")
                nc.vector.tensor_copy(col[:], ps[:])
                return col

            iD_col = carry_col(aD_row, "D")
            iW_col = carry_col(aW_row, "W")

            segD = small.tile([16, 128], f32)
            nc.vector.scalar_tensor_tensor(
                out=segD[:], in0=Pm[:], scalar=iD_col[:], in1=segD0[:],
                op0=Alu.mult, op1=Alu.add)
            segW = small.tile([16, 128], f32)
            nc.vector.scalar_tensor_tensor(
                out=segW[:], in0=Pm[:], scalar=iW_col[:], in1=segW0[:],
                op0=Alu.mult, op1=Alu.add)

            # ---------- final reductions ----------
            Dl = small.tile([16, 128], f32)
            nc.vector.tensor_tensor(out=Dl[:], in0=segD[:], in1=l_td[:],
                                    op=Alu.mult)
            Wl = small.tile([16, 128], f32)
            nc.gpsimd.tensor_tensor(out=Wl[:], in0=segW[:], in1=l_td[:],
                                    op=Alu.mult)

            acc5 = small.tile([16, 5], f32)
            scr16 = small.tile([16, 128], f32)
            nc.vector.tensor_scalar(out=scr16[:], in0=Wl[:], scalar1=1.0,
                                    scalar2=0.0, op0=Alu.mult, op1=Alu.add,
                                    accum_out=acc5[:, 0:1])
            nc.vector.scalar_tensor_tensor(
                out=scr16[:], in0=Dl[:], scalar=0.0, in1=Dl[:],
                op0=Alu.add, op1=Alu.mult, accum_out=acc5[:, 1:2])
            num = small.tile([16, 128], f32)
            nc.vector.tensor_tensor(out=num[:], in0=Dl[:], in1=Wl[:],
                                    op=Alu.subtract)
            den = small.tile([16, 128], f32)
            nc.vector.tensor_tensor(out=den[:], in0=Dl[:], in1=Wl[:],
                                    op=Alu.add)
            lz = small.tile([16, 128], f32)
            nc.vector.tensor_scalar(out=lz[:], in0=l_td[:],
                                    scalar1=(EPS_COND - 1.0), scalar2=1.0,
                                    op0=Alu.mult, op1=Alu.add)
            nc.vector.tensor_tensor(out=den[:], in0=den[:], in1=lz[:],
                                    op=Alu.add)
            rden = small.tile([16, 128], f32)
            nc.vector.reciprocal(rden[:], den[:])
            nc.vector.scalar_tensor_tensor(
                out=scr16[:], in0=num[:], scalar=0.0, in1=rden[:],
                op0=Alu.add, op1=Alu.mult, accum_out=acc5[:, 2:3])
            nc.vector.tensor_scalar(out=scr16[:], in0=l_td[:], scalar1=1.0,
                                    scalar2=0.0, op0=Alu.mult, op1=Alu.add,
                                    accum_out=acc5[:, 3:4])
            nc.vector.tensor_scalar(out=scr16[:], in0=deg[:], scalar1=1.0,
                                    scalar2=0.0, op0=Alu.mult, op1=Alu.add,
                                    accum_out=acc5[:, 4:5])

            a5_ps = psm.tile([5, 16], f32, tag="sm")
            nc.tensor.transpose(a5_ps[:], acc5[:], ident[:16, :16])
            a5T = small.tile([5, 16], f32)
            nc.vector.tensor_copy(a5T[:], a5_ps[:])
            sums5 = small.tile([5, 1], f32)
            nc.vector.tensor_reduce(out=sums5[:], in_=a5T[:],
                                    axis=mybir.AxisListType.X, op=Alu.add)
            s5_ps = psm.tile([1, 5], f32, tag="sm")
            nc.tensor.transpose(s5_ps[:], sums5[:], ident[:5, :5])
            srow = small.tile([1, 5], f32)
            nc.vector.tensor_copy(srow[:], s5_ps[:])

            # srow = [W_sum, Dsq, cond_sum, n_comms, total]
            rtot = small.tile([1, 1], f32)
            nc.vector.reciprocal(rtot[:], srow[:, 4:5])
            t1 = small.tile([1, 1], f32)
            nc.vector.tensor_tensor(out=t1[:], in0=srow[:, 1:2], in1=rtot[:],
                                    op=Alu.mult)
            modn = small.tile([1, 1], f32)
            nc.vector.tensor_tensor(out=modn[:], in0=srow[:, 0:1], in1=t1[:],
                                    op=Alu.subtract)
            out_s = small.tile([1, 2], f32)
            nc.vector.tensor_tensor(out=out_s[:, 0:1], in0=modn[:],
                                    in1=rtot[:], op=Alu.mult)
            ncc = small.tile([1, 1], f32)
            nc.vector.tensor_scalar(out=ncc[:], in0=srow[:, 3:4], scalar1=1.0,
                                    scalar2=None, op0=Alu.max)
            rncc = small.tile([1, 1], f32)
            nc.vector.reciprocal(rncc[:], ncc[:])
            nc.vector.tensor_tensor(out=out_s[:, 1:2], in0=srow[:, 2:3],
                                    in1=rncc[:], op=Alu.mult)

            nc.sync.dma_start(out_d[:], out_s[:])

    nc.compile()
    return nc


def _get_compiled():
    global _COMPILED
    if _COMPILED is None:
        _COMPILED = _build()
    return _COMPILED


def _run(representations, boundaries, trace=False):
    from concourse.bass_utils import run_bass_kernel_spmd
    nc = _get_compiled()
    B = representations.shape[0]
    in_maps = [
        {"rep": np.ascontiguousarray(representations[i], dtype=np.float32),
         "bnd": np.ascontiguousarray(boundaries[i], dtype=np.int32)}
        for i in range(B)
    ]
    res = run_bass_kernel_spmd(nc, in_maps, list(range(B)), trace=trace)
    out = np.stack([res.results[i]["out"][0] for i in range(B)], axis=1)
    return out.astype(np.float32), res


def kernel(representations, boundaries):
    out, _ = _run(np.asarray(representations), np.asarray(boundaries))
    return out
